# revision 5
# baseline (speedup 1.0000x reference)
"""Trainium2 Bass kernel for nn_BCE_Loss (retrieval_knn).

Distributed strategy (8 NeuronCores, SPMD):
  - Host prepares the L2-normalized embedding matrix once (f32 math, bf16
    cast) in transposed layout, and row-stripe shards the WORK in global
    order: core c computes similarity rows [c*1024, (c+1)*1024).
    Per-core inputs:
      xto [4, 128, 1024] bf16 -- the core's own 1024 columns of x-hat^T
                                 (lhsT chunks, d-major),
      xta [8, 4, 128, 1024] bf16 -- all 8192 columns (rhs chunks; same
                                 array on every core).
    Host prep replaces the all-gather of the sharding hint: collectives in
    this environment run at ~0.4 GB/s, while input DMA streams at full HBM
    bandwidth and overlaps with compute.
  - Device (per core): the [1024, 8192] cosine stripe is computed tile-by-
    tile through PSUM (bf16 matmul, f32 accumulate), 8 scan blocks of 1024
    columns per 128-row tile. Per block, top-8 per row is extracted either:
      P1: DVE max8 + max_index directly on PSUM (no evacuation), or
      P3: ACT evacuates PSUM with fused magic rounding (t = v*2^24 + 1.5*2^36
          snaps v*2^24 to the 2^13 grid), GPSIMD packs the column id into the
          value ((t - BIG) + iota, both adds exact in f32), DVE max8.
    The P1/P3 split balances the scan work across DVE / ACT / Pool.
  - The 64 candidates per row are packed as p = round(v*2048)*8192 + gcol
    (value and global column share one f32) and merged with 4 x (max8 +
    match_replace) into sorted top-32 packed values.
  - Host: decode (value, column), drop the self-match (col == row or v > 0.9),
    gather labels, and compute the BCE loss (tiny: 8192 x k).

Self-exclusion: cos(self) ~ 1.0 is always the global row max, so instead of
masking the diagonal on device, the kernel returns top-32 and the host drops
the self entry -- the SPMD program is identical across cores with no
core-dependent diagonal offset.
"""

from contextlib import ExitStack

import numpy as np

import concourse.bass as bass
import concourse.mybir as mybir
import concourse.tile as tile
from concourse.bass import ts
from concourse.bass_utils import run_bass_kernel_spmd
from concourse.vector_clock import ScopedClock, VectorClock

F32 = mybir.dt.float32
BF16 = mybir.dt.bfloat16
U32 = mybir.dt.uint32
I32 = mybir.dt.int32
AF = mybir.ActivationFunctionType
ALU = mybir.AluOpType

B, D = 8192, 512
M = 8              # cores
BL = B // M        # 1024 rows per core
NRT = BL // 128    # 8 row tiles per core
NSB = 8            # 8 scan blocks of 1024 columns
MAGIC = 12582912.0        # 1.5 * 2**23: add+subtract rounds to nearest int
BIGMAGIC = 103079215104.0  # 1.5 * 2**36: rounds v*2^24 to multiples of 2^13
NEG = -3.0e38
OUTW = 32          # top-32 out (top-k + self + slack)

# scan blocks 0..N_P1-1 use the P1 (DVE-only, PSUM-direct) path; the rest use
# P3 (ACT evac+round -> Pool pack -> DVE max8). Tuned for engine balance.
N_P1 = 8
SCAN_SBUF = True


# ---------------------------------------------------------------------------
# Environment workarounds: this container's walrus accepts at most ONE sem
# wait per instruction, and its runtime crashes on the explicit EventSemaphore
# butterfly barrier TileContext emits at its tail.
# ---------------------------------------------------------------------------

def _patched_drain_and_barrier(self, tick_clock, wait_clock):
    nc = self.nc
    vc = tick_clock.global_clock
    n = len(vc)
    for p in range(n):
        t = vc[p]
        if t > 0:
            pvc = VectorClock([0] * n)
            pvc.require_at_least(p, t)
            nop = nc.sync.nop()
            wait_clock.add_sem_waits(nop.ins, ScopedClock({None: pvc}))
    nc.sync.drain()
    nc._nrt_pseudo_barrier()
    assert self.sems is not None
    popped = nc._tile_sem_poison_stack.pop()
    assert popped is self._sem_poison
    nc.clear_and_free_semaphores(list(self.sems.allocated().values()))
    nc._nrt_pseudo_barrier()


tile.TileContext._drain_and_barrier = _patched_drain_and_barrier


def _split_multi_waits(nc):
    import bass_rust

    for f in nc.m.functions:
        for bb in f.blocks:
            out = []
            changed = False
            for ins in bb.instructions:
                si = ins.sync_info
                waits = list(si.on_wait) if si is not None else []
                if len(waits) > 1:
                    changed = True
                    for w in waits[:-1]:
                        nop = mybir.InstNoOp(
                            name=f"I-wsplit-{nc.next_id()}", ins=[], outs=[]
                        )
                        nop.engine = ins.engine
                        nop.sync_info = bass_rust.SyncInfo(on_wait=[w], on_update=[])
                        out.append(nop)
                    ins.sync_info = bass_rust.SyncInfo(
                        on_wait=[waits[-1]], on_update=list(si.on_update)
                    )
                out.append(ins)
            if changed:
                bb.instructions = out


# ---------------------------------------------------------------------------
# Kernel build
# ---------------------------------------------------------------------------

def build_nc(repeat=1):
    nc = bass.Bass(num_devices=M)
    xto = nc.declare_dram_parameter("xto", [4, 128, 1024], BF16, isOutput=False)
    xta = nc.declare_dram_parameter("xta", [M, 4, 128, 1024], BF16,
                                    isOutput=False)
    out = nc.declare_dram_parameter("out", [BL, OUTW], F32, isOutput=True)
    for _rep in range(repeat):
        _build_body(nc, xto, xta, out)
    _split_multi_waits(nc)
    return nc


def _build_body(nc, xto, xta, out):
    n_p1 = N_P1
    n_p3 = NSB - n_p1
    with tile.TileContext(nc) as tc, ExitStack() as octx:
        cpool = octx.enter_context(tc.tile_pool(name="const", bufs=1))
        if n_p3 > 0:
            # iota 0..1023 (local column within a scan block), f32
            iota_i = cpool.tile([128, 1024], I32, name="iota_i")
            nc.gpsimd.iota(iota_i[:], pattern=[[1, 1024]], base=0,
                           channel_multiplier=0)
            iota_f = cpool.tile([128, 1024], F32, name="iota_f")
            nc.scalar.copy(iota_f[:], iota_i[:])
            # P3 candidate slots: global column base for s in [8*n_p1, 64)
            offp3_i = cpool.tile([128, 8 * n_p3], I32, name="offp3_i")
            nc.gpsimd.iota(offp3_i[:], pattern=[[1024, n_p3], [0, 8]],
                           base=1024 * n_p1, channel_multiplier=0)
            offp3 = cpool.tile([128, 8 * n_p3], F32, name="offp3")
            nc.scalar.copy(offp3[:], offp3_i[:])
        if n_p1 > 0:
            # P1 candidate slots: global column base 1024*(s//8)
            offp1_i = cpool.tile([128, 8 * n_p1], I32, name="offp1_i")
            nc.gpsimd.iota(offp1_i[:], pattern=[[1024, n_p1], [0, 8]], base=0,
                           channel_multiplier=0)
            offp1 = cpool.tile([128, 8 * n_p1], F32, name="offp1")
            nc.scalar.copy(offp1[:], offp1_i[:])

        xt_own_pool = octx.enter_context(tc.tile_pool(name="xto", bufs=1))
        xt_own = xt_own_pool.tile([128, 4, 1024], BF16, tag="xt_own",
                                  name="xt_own")
        xt_all_pool = octx.enter_context(tc.tile_pool(name="xta", bufs=1))
        xt_all = [
            xt_all_pool.tile([128, 4, 1024], BF16, tag=f"xta{i}", name=f"xta{i}")
            for i in range(M)
        ]

        mm = octx.enter_context(tc.tile_pool(name="mm", bufs=4, space="PSUM"))
        sb = octx.enter_context(tc.tile_pool(name="sb", bufs=4))
        cand = octx.enter_context(tc.tile_pool(name="cand", bufs=1))
        fin = octx.enter_context(tc.tile_pool(name="fin", bufs=2))

        # ---- load inputs (xta chunk j gates scan block j; all overlap compute)
        for d4 in range(4):
            nc.sync.dma_start(xt_own[:, d4, :], xto[d4, :, :])
        for i in range(M):
            for d4 in range(4):
                nc.sync.dma_start(xt_all[i][:, d4, :], xta[i, d4, :, :])

        # ---- Phase 2: stripe matmul + per-block top-8
        vals = [
            cand.tile([128, 8 * max(n_p1, 1)], F32, tag=f"VALS{m}",
                      name=f"VALS{m}")
            for m in range(NRT)
        ]
        idx = [
            cand.tile([128, 8 * max(n_p1, 1)], U32, tag=f"IDX{m}",
                      name=f"IDX{m}")
            for m in range(NRT)
        ]
        pk = [
            cand.tile([128, 64], F32, tag=f"PK{m}", name=f"PK{m}")
            for m in range(NRT)
        ]

        def do_block(m, j):
            ps = mm.tile([128, 1024], F32, tag="ps", name=f"ps_{m}_{j}")
            for d4 in range(4):
                lhsT = xt_own[:, d4, ts(m, 128)]
                for h in range(2):
                    nc.tensor.matmul(
                        ps[:, ts(h, 512)], lhsT,
                        xt_all[j][:, d4, ts(h, 512)],
                        start=(d4 == 0), stop=(d4 == 3),
                    )
            if j < n_p1:
                if SCAN_SBUF:
                    t = sb.tile([128, 1024], F32, tag="t")
                    nc.scalar.copy(t[:], ps[:])
                    nc.vector.max(vals[m][:, ts(j, 8)], t[:])
                    nc.vector.max_index(idx[m][:, ts(j, 8)],
                                        vals[m][:, ts(j, 8)], t[:])
                else:
                    nc.vector.max(vals[m][:, ts(j, 8)], ps[:])
                    nc.vector.max_index(idx[m][:, ts(j, 8)],
                                        vals[m][:, ts(j, 8)], ps[:])
            else:
                t = sb.tile([128, 1024], F32, tag="t")
                nc.scalar.activation(t[:], ps[:], AF.Copy,
                                     scale=16777216.0, bias=BIGMAGIC)
                pc = sb.tile([128, 1024], F32, tag="pc")
                nc.gpsimd.scalar_tensor_tensor(
                    pc[:], in0=t[:], scalar=BIGMAGIC, in1=iota_f[:],
                    op0=ALU.subtract, op1=ALU.add,
                )
                nc.vector.max(pk[m][:, ts(j, 8)], pc[:])

        def do_merge(m):
            # P1 candidates: pack value+index -> pk[m][:, 0:8*n_p1]
            if n_p1 > 0:
                w = 8 * n_p1
                vq = fin.tile([128, w], F32, tag="vq")
                nc.scalar.activation(vq[:], vals[m][:], AF.Copy,
                                     scale=2048.0, bias=MAGIC)
                q = fin.tile([128, w], F32, tag="q")
                nc.vector.tensor_scalar_add(q[:], vq[:], -MAGIC)
                idxf = fin.tile([128, w], F32, tag="idxf")
                nc.scalar.copy(idxf[:], idx[m][:])
                t1 = fin.tile([128, w], F32, tag="t1")
                nc.vector.tensor_tensor(t1[:], idxf[:], offp1[:], op=ALU.add)
                nc.vector.scalar_tensor_tensor(
                    pk[m][:, 0:w], in0=q[:], scalar=8192.0, in1=t1[:],
                    op0=ALU.mult, op1=ALU.add,
                )
            # P3 candidates: add global column base in place
            if n_p1 < NSB:
                w = 8 * n_p1
                nc.vector.tensor_tensor(pk[m][:, w:64], pk[m][:, w:64],
                                        offp3[:], op=ALU.add)
            pv = fin.tile([128, OUTW], F32, tag="pv")
            p1t = fin.tile([128, 64], F32, tag="p1")
            p2t = fin.tile([128, 64], F32, tag="p2")
            p3t = fin.tile([128, 64], F32, tag="p3")
            nc.vector.max(pv[:, 0:8], pk[m][:])
            nc.vector.match_replace(p1t[:], pv[:, 0:8], pk[m][:], NEG)
            nc.vector.max(pv[:, 8:16], p1t[:])
            nc.vector.match_replace(p2t[:], pv[:, 8:16], p1t[:], NEG)
            nc.vector.max(pv[:, 16:24], p2t[:])
            nc.vector.match_replace(p3t[:], pv[:, 16:24], p2t[:], NEG)
            nc.vector.max(pv[:, 24:32], p3t[:])
            nc.sync.dma_start(out[ts(m, 128), :], pv[:])

        for j in range(NSB):
            for m in range(NRT):
                do_block(m, j)
        for m in range(NRT):
            do_merge(m)


_NC = None


def _get_nc():
    global _NC
    if _NC is None:
        _NC = build_nc()
    return _NC


def prep_inputs(x32):
    """Host prep: L2-normalize rows (f32), cast bf16, lay out transposed
    d-major chunks. Returns (xto_per_core list, xta shared)."""
    import ml_dtypes

    norm = np.maximum(np.sqrt((x32.astype(np.float64) ** 2).sum(axis=1)),
                      1e-12)
    xn = (x32 / norm[:, None].astype(np.float32)).astype(ml_dtypes.bfloat16)
    # xta[i, d4, p, c] = xn[i*1024 + c, d4*128 + p]
    xta = np.ascontiguousarray(
        xn.reshape(M, 1024, 4, 128).transpose(0, 2, 3, 1)
    )
    xtos = [np.ascontiguousarray(xta[c]) for c in range(M)]
    return xtos, xta


def make_in_maps(x32):
    xtos, xta = prep_inputs(x32)
    return [{"xto": xtos[c], "xta": xta} for c in range(M)]


def run_device(x32, trace=False, **kwargs):
    """Run the SPMD kernel; returns (pv [B, OUTW] f32, BassKernelResults)."""
    nc = _get_nc()
    in_maps = make_in_maps(x32)
    res = run_bass_kernel_spmd(nc, in_maps, core_ids=list(range(M)),
                               trace=trace, **kwargs)
    pv = np.concatenate([res.results[c]["out"] for c in range(M)], axis=0)
    return pv, res


def decode_loss(pv, labels, k):
    """Decode packed top-32 -> (values, global column ids) -> BCE loss."""
    pv64 = pv.astype(np.float64)
    q = np.floor(pv64 / 8192.0)
    col = (pv64 - q * 8192.0).astype(np.int64)        # global column in [0, B)
    vhat = q / 2048.0                                 # quantized cosine
    rows = np.arange(B)[:, None]
    # drop the self entry (col == row, or value ~1.0 if the col bit was lost)
    valid = (col != rows) & (vhat <= 0.9)
    order = np.argsort(~valid, axis=1, kind="stable")  # valid first, desc order
    take = order[:, :k]
    vk = np.take_along_axis(vhat, take, axis=1)
    ck = np.take_along_axis(col, take, axis=1)
    preds = (vk + 1.0) * 0.5
    t = (labels[ck] == labels[:, None]).astype(np.float64)
    logp = np.maximum(np.log(np.maximum(preds, 1e-300)), -100.0)
    log1mp = np.maximum(np.log1p(-np.minimum(preds, 1.0 - 1e-16)), -100.0)
    loss = -(t * logp + (1.0 - t) * log1mp)
    return np.float32(loss.mean())


def kernel(batch, labels, k):
    k = int(k)
    assert 0 < k <= OUTW - 1, f"kernel supports k <= {OUTW - 1}, got {k}"
    x32 = np.asarray(batch, dtype=np.float32)
    assert x32.shape == (B, D)
    labels = np.asarray(labels)
    pv, _ = run_device(x32)
    return decode_loss(pv, labels, k)


# revision 9
# speedup vs baseline: 10.9851x; 10.9851x over previous
"""Trainium2 Bass kernel for nn_BCE_Loss (retrieval_knn).

Distributed strategy (8 NeuronCores, SPMD):
  - Host prepares the L2-normalized embedding matrix once (f32 math), then
    quantizes it to INTEGERS: xq = round(xhat * 128), stored bf16 (integers
    <= 2^8 are bf16-exact). Work is row-stripe sharded in global order:
    core c computes similarity rows [c*1024, (c+1)*1024).
    Per-core inputs:
      xto [4, 128, 1024] bf16 -- the core's own 1024 columns of xq^T
                                 (lhsT chunks, d-major),
      xta [8, 4, 128, 1024] bf16 -- all 8192 columns (rhs chunks; same
                                 array on every core).
    Host prep replaces the all-gather of the sharding hint: collectives in
    this environment run at ~0.4 GB/s, while input DMA overlaps with compute.
  - Device (per core): the [1024, 8192] integer Gram stripe m = xq xq^T is
    computed through PSUM (bf16 matmul, f32 accumulate). Because all products
    and partial sums are integers < 2^24, the accumulation is EXACT. A fifth
    rank-2 matmul per tile adds iota_c * 2^-10 (split into two bf16-exact
    rows of 5 bits each), embedding the in-block column id in the fraction:
        psum[r, c] = m[r, c] + c * 2^-10   (exact in f32 for |m| < 2^13)
    One DVE max8 per [128, 1024] scan block then yields the top-8
    (value, column) pairs per row in a single pass -- no max_index, no
    pack, no evacuation, no on-device merge.
  - Output: the 64 candidates per row (8 scan blocks x top-8). Host decodes
    m = floor(s), col = frac(s)*1024 + 1024*(slot//8), v = m/2^14, drops the
    self-match (col == row or v > 0.9), takes top-k, gathers labels, and
    computes the BCE loss (tiny: 8192 x 64).

Self-exclusion: cos(self) ~ 1.0 is always the global row max, so instead of
masking the diagonal on device the host drops it -- the SPMD program is
identical across cores with no core-dependent diagonal offset.

Accuracy: quantizing xhat to 2^-7 absolute adds cosine noise sigma ~ 3.2e-3
(like fp8) and values are read back at the same precision; the resulting
loss error is ~1e-4 relative, far inside the 2e-2 gate.
"""

from contextlib import ExitStack

import numpy as np

import concourse.bass as bass
import concourse.mybir as mybir
import concourse.tile as tile
from concourse.bass import ts
from concourse.bass_utils import run_bass_kernel_spmd
from concourse.vector_clock import ScopedClock, VectorClock

F32 = mybir.dt.float32
BF16 = mybir.dt.bfloat16
U32 = mybir.dt.uint32
I32 = mybir.dt.int32
AF = mybir.ActivationFunctionType
ALU = mybir.AluOpType

B, D = 8192, 512
M = 8              # cores
BL = B // M        # 1024 rows per core
NRT = BL // 128    # 8 row tiles per core
NSB = 8            # 8 scan blocks of 1024 columns
QS = 128.0         # quantization scale: xq = round(xhat * QS)
VSCALE = QS * QS   # m = cos * VSCALE
OUTW = 64          # 8 blocks x top-8 candidates per row


# ---------------------------------------------------------------------------
# Environment workarounds: this container's walrus accepts at most ONE sem
# wait per instruction, and its runtime crashes on the explicit EventSemaphore
# butterfly barrier TileContext emits at its tail.
# ---------------------------------------------------------------------------

def _patched_drain_and_barrier(self, tick_clock, wait_clock):
    nc = self.nc
    vc = tick_clock.global_clock
    n = len(vc)
    for p in range(n):
        t = vc[p]
        if t > 0:
            pvc = VectorClock([0] * n)
            pvc.require_at_least(p, t)
            nop = nc.sync.nop()
            wait_clock.add_sem_waits(nop.ins, ScopedClock({None: pvc}))
    nc.sync.drain()
    nc._nrt_pseudo_barrier()
    assert self.sems is not None
    popped = nc._tile_sem_poison_stack.pop()
    assert popped is self._sem_poison
    nc.clear_and_free_semaphores(list(self.sems.allocated().values()))
    nc._nrt_pseudo_barrier()


tile.TileContext._drain_and_barrier = _patched_drain_and_barrier


def _split_multi_waits(nc):
    import bass_rust

    for f in nc.m.functions:
        for bb in f.blocks:
            out = []
            changed = False
            for ins in bb.instructions:
                si = ins.sync_info
                waits = list(si.on_wait) if si is not None else []
                if len(waits) > 1:
                    changed = True
                    for w in waits[:-1]:
                        nop = mybir.InstNoOp(
                            name=f"I-wsplit-{nc.next_id()}", ins=[], outs=[]
                        )
                        nop.engine = ins.engine
                        nop.sync_info = bass_rust.SyncInfo(on_wait=[w], on_update=[])
                        out.append(nop)
                    ins.sync_info = bass_rust.SyncInfo(
                        on_wait=[waits[-1]], on_update=list(si.on_update)
                    )
                out.append(ins)
            if changed:
                bb.instructions = out


# ---------------------------------------------------------------------------
# Kernel build
# ---------------------------------------------------------------------------

def build_nc(repeat=1):
    nc = bass.Bass(num_devices=M)
    xto = nc.declare_dram_parameter("xto", [4, 128, 1024], BF16, isOutput=False)
    xta = nc.declare_dram_parameter("xta", [M, 4, 128, 1024], BF16,
                                    isOutput=False)
    iot = nc.declare_dram_parameter("iot", [2, 1024], BF16, isOutput=False)
    one = nc.declare_dram_parameter("one", [2, 128], BF16, isOutput=False)
    out = nc.declare_dram_parameter("out", [BL, OUTW], F32, isOutput=True)
    for _rep in range(repeat):
        _build_body(nc, xto, xta, iot, one, out)
    _split_multi_waits(nc)
    return nc


def _build_body(nc, xto, xta, iot, one, out):
    with tile.TileContext(nc) as tc, ExitStack() as octx:
        cpool = octx.enter_context(tc.tile_pool(name="const", bufs=1))
        # iota fraction rows (host-built): row0 = (c>>5)*2^-5,
        # row1 = (c&31)*2^-10 -- bf16-exact; their sum is c*2^-10.
        iota2 = cpool.tile([2, 1024], BF16, name="iota2")
        nc.sync.dma_start(iota2[:], iot[:, :])
        ones2 = cpool.tile([2, 128], BF16, name="ones2")
        nc.sync.dma_start(ones2[:], one[:, :])

        xt_own_pool = octx.enter_context(tc.tile_pool(name="xto", bufs=1))
        xt_own = xt_own_pool.tile([128, 4, 1024], BF16, tag="xt_own",
                                  name="xt_own")
        xt_all_pool = octx.enter_context(tc.tile_pool(name="xta", bufs=1))
        xt_all = [
            xt_all_pool.tile([128, 4, 1024], BF16, tag=f"xta{i}", name=f"xta{i}")
            for i in range(M)
        ]

        mm = octx.enter_context(tc.tile_pool(name="mm", bufs=4, space="PSUM"))
        cand = octx.enter_context(tc.tile_pool(name="cand", bufs=1))

        # ---- load inputs (xta chunk j gates scan block j; overlaps compute)
        for d4 in range(4):
            nc.sync.dma_start(xt_own[:, d4, :], xto[d4, :, :])
        for i in range(M):
            for d4 in range(4):
                nc.sync.dma_start(xt_all[i][:, d4, :], xta[i, d4, :, :])

        # ---- integer Gram stripe + iota fraction + per-block top-8
        cands = [
            cand.tile([128, OUTW], F32, tag=f"C{m}", name=f"C{m}")
            for m in range(NRT)
        ]

        def do_block(m, j):
            ps = mm.tile([128, 1024], F32, tag="ps", name=f"ps_{m}_{j}")
            for d4 in range(4):
                lhsT = xt_own[:, d4, ts(m, 128)]
                for h in range(2):
                    nc.tensor.matmul(
                        ps[:, ts(h, 512)], lhsT,
                        xt_all[j][:, d4, ts(h, 512)],
                        start=(d4 == 0), stop=False,
                    )
            for h in range(2):
                nc.tensor.matmul(
                    ps[:, ts(h, 512)], ones2[:, :],
                    iota2[:, ts(h, 512)],
                    start=False, stop=True,
                )
            nc.vector.max(cands[m][:, ts(j, 8)], ps[:])

        for j in range(NSB):
            for m in range(NRT):
                do_block(m, j)
        for m in range(NRT):
            nc.sync.dma_start(out[ts(m, 128), :], cands[m][:])


_NC = None


def _get_nc():
    global _NC
    if _NC is None:
        _NC = build_nc()
    return _NC


def prep_inputs(x32):
    """Host prep: L2-normalize rows (f32), quantize to integers * 2^-7,
    lay out transposed d-major chunks. Returns (xto per core, xta shared)."""
    import ml_dtypes

    norm = np.maximum(np.sqrt((x32.astype(np.float64) ** 2).sum(axis=1)),
                      1e-12)
    xn = x32 / norm[:, None].astype(np.float32)
    xq = np.rint(xn * QS).astype(np.float32).astype(ml_dtypes.bfloat16)
    # xta[i, d4, p, c] = xq[i*1024 + c, d4*128 + p]
    xta = np.ascontiguousarray(
        xq.reshape(M, 1024, 4, 128).transpose(0, 2, 3, 1)
    )
    xtos = [np.ascontiguousarray(xta[c]) for c in range(M)]
    return xtos, xta


def make_in_maps(x32):
    import ml_dtypes

    xtos, xta = prep_inputs(x32)
    c = np.arange(1024)
    iot = np.stack([(c >> 5) * 2.0 ** -5, (c & 31) * 2.0 ** -10]).astype(
        ml_dtypes.bfloat16)
    one = np.ones((2, 128), ml_dtypes.bfloat16)
    return [{"xto": xtos[c2], "xta": xta, "iot": iot, "one": one}
            for c2 in range(M)]


def run_device(x32, trace=False, **kwargs):
    """Run the SPMD kernel; returns (pv [B, OUTW] f32, BassKernelResults)."""
    nc = _get_nc()
    in_maps = make_in_maps(x32)
    res = run_bass_kernel_spmd(nc, in_maps, core_ids=list(range(M)),
                               trace=trace, **kwargs)
    pv = np.concatenate([res.results[c]["out"] for c in range(M)], axis=0)
    return pv, res


def decode_loss(pv, labels, k):
    """Decode candidates s = m + c*2^-10 -> (cosine, global column) -> BCE."""
    s = pv.astype(np.float64)
    mm_ = np.floor(s)
    cloc = np.rint((s - mm_) * 1024.0).astype(np.int64)
    blk = (np.arange(OUTW)[None, :] // 8) * 1024
    col = np.clip(cloc + blk, 0, B - 1)
    vhat = mm_ / VSCALE
    rows = np.arange(B)[:, None]
    valid = (col != rows) & (vhat <= 0.9)
    # rank candidates per row by value, valid first
    key = np.where(valid, vhat, -1e30)
    order = np.argsort(-key, axis=1, kind="stable")
    take = order[:, :k]
    vk = np.take_along_axis(vhat, take, axis=1)
    ck = np.take_along_axis(col, take, axis=1)
    preds = np.clip((vk + 1.0) * 0.5, 1e-12, 1.0 - 1e-16)
    t = (labels[ck] == labels[:, None]).astype(np.float64)
    logp = np.maximum(np.log(preds), -100.0)
    log1mp = np.maximum(np.log1p(-preds), -100.0)
    loss = -(t * logp + (1.0 - t) * log1mp)
    return np.float32(loss.mean())


def kernel(batch, labels, k):
    k = int(k)
    assert 0 < k <= 24, f"kernel supports k <= 24, got {k}"
    x32 = np.asarray(batch, dtype=np.float32)
    assert x32.shape == (B, D)
    labels = np.asarray(labels)
    pv, _ = run_device(x32)
    return decode_loss(pv, labels, k)


# revision 15
# speedup vs baseline: 15.5168x; 1.4125x over previous
"""Trainium2 Bass kernel for nn_BCE_Loss (retrieval_knn).

Distributed strategy (8 NeuronCores, SPMD):
  - Host prepares the L2-normalized embedding matrix once (f32 math), then
    quantizes it to INTEGERS: xq = round(xhat * 128), stored bf16 (integers
    <= 2^8 are bf16-exact). Work is row-stripe sharded in global order:
    core c computes similarity rows [c*1024, (c+1)*1024).
    Per-core inputs:
      xto [4, 128, 1024] bf16 -- the core's own 1024 columns of xq^T
                                 (lhsT chunks, d-major),
      xta [8, 4, 128, 1024] bf16 -- all 8192 columns (rhs chunks; same
                                 array on every core).
    Host prep replaces the all-gather of the sharding hint: collectives in
    this environment run at ~0.4 GB/s, while input DMA overlaps with compute.
  - Device (per core): the [1024, 8192] integer Gram stripe m = xq xq^T is
    computed through PSUM (bf16 matmul, f32 accumulate). Because all products
    and partial sums are integers < 2^24, the accumulation is EXACT. A fifth
    rank-2 matmul per tile adds iota_c * 2^-10 (split into two bf16-exact
    rows of 5 bits each), embedding the in-block column id in the fraction:
        psum[r, c] = m[r, c] + c * 2^-10   (exact in f32 for |m| < 2^13)
    One DVE max8 per [128, 1024] scan block then yields the top-8
    (value, column) pairs per row in a single pass -- no max_index, no
    pack, no evacuation, no on-device merge.
  - Output: the 64 candidates per row (8 scan blocks x top-8). Host decodes
    m = floor(s), col = frac(s)*1024 + 1024*(slot//8), v = m/2^14, drops the
    self-match (col == row or v > 0.9), takes top-k, gathers labels, and
    computes the BCE loss (tiny: 8192 x 64).

Self-exclusion: cos(self) ~ 1.0 is always the global row max, so instead of
masking the diagonal on device the host drops it -- the SPMD program is
identical across cores with no core-dependent diagonal offset.

Accuracy: quantizing xhat to 2^-7 absolute adds cosine noise sigma ~ 3.2e-3
(like fp8) and values are read back at the same precision; the resulting
loss error is ~1e-4 relative, far inside the 2e-2 gate.
"""

from contextlib import ExitStack

import numpy as np

import concourse.bass as bass
import concourse.mybir as mybir
import concourse.tile as tile
from concourse.bass import ts
from concourse.bass_utils import run_bass_kernel_spmd
from concourse.vector_clock import ScopedClock, VectorClock

F32 = mybir.dt.float32
BF16 = mybir.dt.bfloat16
U32 = mybir.dt.uint32
I32 = mybir.dt.int32
AF = mybir.ActivationFunctionType
ALU = mybir.AluOpType

B, D = 8192, 512
M = 8              # cores
BL = B // M        # 1024 rows per core
NRT = BL // 128    # 8 row tiles per core
NSB = 8            # 8 scan blocks of 1024 columns
QS = 128.0         # quantization scale: xq = round(xhat * QS)
VSCALE = QS * QS   # m = cos * VSCALE
OUTW = 64          # 8 blocks x top-8 candidates per row
GRP = 4            # scan blocks per weight-reuse group (4 x [128,1024] PSUM)
# every ACT_INIT_MOD-th group gets its iota fraction pre-written into PSUM by
# the (otherwise idle) ACT engine instead of the two extra PE matmuls.
# 0 = never (all PE).
ACT_INIT_MOD = 0


# ---------------------------------------------------------------------------
# Environment workarounds: this container's walrus accepts at most ONE sem
# wait per instruction, and its runtime crashes on the explicit EventSemaphore
# butterfly barrier TileContext emits at its tail.
# ---------------------------------------------------------------------------

def _patched_drain_and_barrier(self, tick_clock, wait_clock):
    nc = self.nc
    vc = tick_clock.global_clock
    n = len(vc)
    for p in range(n):
        t = vc[p]
        if t > 0:
            pvc = VectorClock([0] * n)
            pvc.require_at_least(p, t)
            nop = nc.sync.nop()
            wait_clock.add_sem_waits(nop.ins, ScopedClock({None: pvc}))
    nc.sync.drain()
    nc._nrt_pseudo_barrier()
    assert self.sems is not None
    popped = nc._tile_sem_poison_stack.pop()
    assert popped is self._sem_poison
    nc.clear_and_free_semaphores(list(self.sems.allocated().values()))
    nc._nrt_pseudo_barrier()


tile.TileContext._drain_and_barrier = _patched_drain_and_barrier


def _split_multi_waits(nc):
    import bass_rust

    for f in nc.m.functions:
        for bb in f.blocks:
            out = []
            changed = False
            for ins in bb.instructions:
                si = ins.sync_info
                waits = list(si.on_wait) if si is not None else []
                if len(waits) > 1:
                    changed = True
                    for w in waits[:-1]:
                        nop = mybir.InstNoOp(
                            name=f"I-wsplit-{nc.next_id()}", ins=[], outs=[]
                        )
                        nop.engine = ins.engine
                        nop.sync_info = bass_rust.SyncInfo(on_wait=[w], on_update=[])
                        out.append(nop)
                    ins.sync_info = bass_rust.SyncInfo(
                        on_wait=[waits[-1]], on_update=list(si.on_update)
                    )
                out.append(ins)
            if changed:
                bb.instructions = out


# ---------------------------------------------------------------------------
# Kernel build
# ---------------------------------------------------------------------------

def build_nc(repeat=1):
    nc = bass.Bass(num_devices=M)
    xto = nc.declare_dram_parameter("xto", [4, 128, 1024], BF16, isOutput=False)
    xta = nc.declare_dram_parameter("xta", [M, 4, 128, 1024], BF16,
                                    isOutput=False)
    iot = nc.declare_dram_parameter("iot", [2, 1024], BF16, isOutput=False)
    one = nc.declare_dram_parameter("one", [2, 128], BF16, isOutput=False)
    iof = nc.declare_dram_parameter("iof", [128, 1024], F32, isOutput=False)
    out = nc.declare_dram_parameter("out", [BL, OUTW], F32, isOutput=True)
    for _rep in range(repeat):
        _build_body(nc, xto, xta, iot, one, iof, out)
    _split_multi_waits(nc)
    return nc


def _build_body(nc, xto, xta, iot, one, iof, out):
    with tile.TileContext(nc) as tc, ExitStack() as octx:
        cpool = octx.enter_context(tc.tile_pool(name="const", bufs=1))
        # iota fraction rows (host-built): row0 = (c>>5)*2^-5,
        # row1 = (c&31)*2^-10 -- bf16-exact; their sum is c*2^-10.
        iota2 = cpool.tile([2, 1024], BF16, name="iota2")
        nc.sync.dma_start(iota2[:], iot[:, :])
        ones2 = cpool.tile([2, 128], BF16, name="ones2")
        nc.sync.dma_start(ones2[:], one[:, :])
        if ACT_INIT_MOD:
            iotaf = cpool.tile([128, 1024], F32, name="iotaf")
            nc.sync.dma_start(iotaf[:], iof[:, :])

        xt_own_pool = octx.enter_context(tc.tile_pool(name="xto", bufs=1))
        xt_own = xt_own_pool.tile([128, 4, 1024], BF16, tag="xt_own",
                                  name="xt_own")
        xt_all_pool = octx.enter_context(tc.tile_pool(name="xta", bufs=1))
        xt_all = [
            xt_all_pool.tile([128, 4, 1024], BF16, tag=f"xta{i}", name=f"xta{i}")
            for i in range(M)
        ]

        mm = octx.enter_context(tc.tile_pool(name="mm", bufs=4, space="PSUM"))
        cand = octx.enter_context(tc.tile_pool(name="cand", bufs=1))

        # ---- load inputs (xta chunk j gates scan block j; overlaps compute)
        for d4 in range(4):
            nc.sync.dma_start(xt_own[:, d4, :], xto[d4, :, :])
        for i in range(M):
            for d4 in range(4):
                nc.sync.dma_start(xt_all[i][:, d4, :], xta[i, d4, :, :])

        # ---- integer Gram stripe + iota fraction + per-block top-8
        cands = [
            cand.tile([128, OUTW], F32, tag=f"C{m}", name=f"C{m}")
            for m in range(NRT)
        ]

        def do_group(m, g, gi):
            js = range(g * GRP, (g + 1) * GRP)
            act_init = bool(ACT_INIT_MOD) and gi % ACT_INIT_MOD == 0
            pss = {}
            for j in js:
                ps = mm.tile([128, 1024], F32, tag="ps", name=f"ps_{m}_{j}")
                pss[j] = ps
                if act_init:
                    nc.scalar.copy(ps[:], iotaf[:])
            # weight-reuse: each lhsT chunk streams all GRP blocks
            for d4 in range(4):
                lhsT = xt_own[:, d4, ts(m, 128)]
                for j in js:
                    for h in range(2):
                        nc.tensor.matmul(
                            pss[j][:, ts(h, 512)], lhsT,
                            xt_all[j][:, d4, ts(h, 512)],
                            start=(d4 == 0 and not act_init),
                            stop=(act_init and d4 == 3),
                            skip_group_check=act_init,
                        )
            if not act_init:
                for j in js:
                    for h in range(2):
                        nc.tensor.matmul(
                            pss[j][:, ts(h, 512)], ones2[:, :],
                            iota2[:, ts(h, 512)],
                            start=False, stop=True,
                        )
            for j in js:
                nc.vector.max(cands[m][:, ts(j, 8)], pss[j][:])

        gi = 0
        for g in range(NSB // GRP):
            for m in range(NRT):
                do_group(m, g, gi)
                gi += 1
        for m in range(NRT):
            nc.sync.dma_start(out[ts(m, 128), :], cands[m][:])


_NC = None


def _get_nc():
    global _NC
    if _NC is None:
        _NC = build_nc()
    return _NC


def prep_inputs(x32):
    """Host prep: L2-normalize rows (f32), quantize to integers * 2^-7,
    lay out transposed d-major chunks. Returns (xto per core, xta shared)."""
    import ml_dtypes

    norm = np.maximum(np.sqrt((x32.astype(np.float64) ** 2).sum(axis=1)),
                      1e-12)
    xn = x32 / norm[:, None].astype(np.float32)
    xq = np.rint(xn * QS).astype(np.float32).astype(ml_dtypes.bfloat16)
    # xta[i, d4, p, c] = xq[i*1024 + c, d4*128 + p]
    xta = np.ascontiguousarray(
        xq.reshape(M, 1024, 4, 128).transpose(0, 2, 3, 1)
    )
    xtos = [np.ascontiguousarray(xta[c]) for c in range(M)]
    return xtos, xta


def make_in_maps(x32):
    import ml_dtypes

    xtos, xta = prep_inputs(x32)
    c = np.arange(1024)
    iot = np.stack([(c >> 5) * 2.0 ** -5, (c & 31) * 2.0 ** -10]).astype(
        ml_dtypes.bfloat16)
    one = np.ones((2, 128), ml_dtypes.bfloat16)
    iof = np.broadcast_to((c * 2.0 ** -10).astype(np.float32),
                          (128, 1024)).copy()
    return [{"xto": xtos[c2], "xta": xta, "iot": iot, "one": one, "iof": iof}
            for c2 in range(M)]


def run_device(x32, trace=False, **kwargs):
    """Run the SPMD kernel; returns (pv [B, OUTW] f32, BassKernelResults)."""
    nc = _get_nc()
    in_maps = make_in_maps(x32)
    res = run_bass_kernel_spmd(nc, in_maps, core_ids=list(range(M)),
                               trace=trace, **kwargs)
    pv = np.concatenate([res.results[c]["out"] for c in range(M)], axis=0)
    return pv, res


def decode_loss(pv, labels, k):
    """Decode candidates s = m + c*2^-10 -> (cosine, global column) -> BCE."""
    s = pv.astype(np.float64)
    mm_ = np.floor(s)
    cloc = np.rint((s - mm_) * 1024.0).astype(np.int64)
    blk = (np.arange(OUTW)[None, :] // 8) * 1024
    col = np.clip(cloc + blk, 0, B - 1)
    vhat = mm_ / VSCALE
    rows = np.arange(B)[:, None]
    valid = (col != rows) & (vhat <= 0.9)
    # rank candidates per row by value, valid first
    key = np.where(valid, vhat, -1e30)
    order = np.argsort(-key, axis=1, kind="stable")
    take = order[:, :k]
    vk = np.take_along_axis(vhat, take, axis=1)
    ck = np.take_along_axis(col, take, axis=1)
    preds = np.clip((vk + 1.0) * 0.5, 1e-12, 1.0 - 1e-16)
    t = (labels[ck] == labels[:, None]).astype(np.float64)
    logp = np.maximum(np.log(preds), -100.0)
    log1mp = np.maximum(np.log1p(-preds), -100.0)
    loss = -(t * logp + (1.0 - t) * log1mp)
    return np.float32(loss.mean())


def kernel(batch, labels, k):
    k = int(k)
    assert 0 < k <= 24, f"kernel supports k <= 24, got {k}"
    x32 = np.asarray(batch, dtype=np.float32)
    assert x32.shape == (B, D)
    labels = np.asarray(labels)
    pv, _ = run_device(x32)
    return decode_loss(pv, labels, k)
